# revision 9
# baseline (speedup 1.0000x reference)
"""DFlashAttention kernel for Trainium2, tensor-parallel across 8 NeuronCores.

Sharding: Megatron-style head parallelism. Core c owns KV head c and Q heads
4c..4c+3 (matches repeat_interleave grouping), i.e. Wq rows [512c, 512c+512),
Wk/Wv rows [128c, 128c+128), Wo columns [512c, 512c+512). Each core computes a
partial output [QL, H] in bf16; the host sums the 8 partials (row-parallel Wo).

v2 layout/schedule notes (all matmuls bf16 except the tiny f32r broadcast):
  - activations/weights fed feature-major (host pre-transposes)
  - Q/K kept d-major bf16 [HD, seq]; scores^T = K-tile.T @ Q per head
  - RoPE: rotate-half done with partition-shifted ACT copies straight out of
    PSUM (no SBUF<->SBUF DMA on the critical path); cos/sin resident in SBUF
  - softmax: per head-pair, both heads' score tiles land in one 2-bank PSUM
    tile and a single wide ACT exp (scale fused) emits bf16 probs for both;
    denominators accumulate on PE with a ones-column stationary at output
    partitions 0/32 (concurrent col-groups); normalization via reciprocal +
    PE broadcast and a DVE multiply
  - V kept k-major bf16 (PE-transposed after the d-major projection); PV
    accumulates attn^T = V^T @ P^T in PSUM over 32 k-tiles
  - Wo: attn^T tiles are the stationary operand; outputs assembled in a
    [128, 4096] bf16 staging row and written as one contiguous 1 MiB DMA on
    the ACT HWDGE ring (separate from the input ring)
"""

import math
from contextlib import ExitStack

import ml_dtypes
import numpy as np

import concourse.bass as bass
import concourse.bacc as bacc
import concourse.mybir as mybir
import concourse.tile as tile
from concourse.bass_utils import run_bass_kernel_spmd

F32 = mybir.dt.float32
F32R = mybir.dt.float32r
BF16 = mybir.dt.bfloat16
AF = mybir.ActivationFunctionType
ALU = mybir.AluOpType

# Full-problem dims (hardcoded per spec)
B, QL, CTX, H = 1, 2048, 2048, 4096
NH, NKV, HD = 32, 8, 128
NCORES = 8
HPC = NH // NKV  # 4 q-heads per core (one KV head per core)


def build_program(ql=QL, ctx_len=CTX, h=H, trace_sim=False, phases="ABC", body_reps=1):
    """Build the per-core Bass program (SPMD: same program, per-core shards)."""
    s = ql + ctx_len          # total kv length
    et = h // 128             # e-tiles (contraction tiles for projections)
    kt = s // 128             # k-tiles in attention
    QC = 512                  # phase A position-chunk
    nch = ql // QC            # chunks (ctx assumed == ql)
    assert ctx_len == ql, "phase A chunking assumes ctx_len == ql"
    QB = 512                  # phase B q-block
    nqb = ql // QB
    scale = 1.0 / math.sqrt(HD)
    DQ = HPC * HD             # 512: per-core q-head dim
    hot = h // 512            # output-column chunks in Wo stage

    nc = bacc.Bacc("TRN2", target_bir_lowering=False, debug=False)

    def din(name, shape, dt_=F32):
        return nc.dram_tensor(name, shape, dt_, kind="ExternalInput").ap()

    hiddenT = din("hiddenT", [h, ql], BF16)       # hidden_states[0].T
    targetT = din("targetT", [h, ctx_len], BF16)  # target_hidden[0].T
    cosT = din("cosT", [HD, s])                   # cos[0].T
    sinT = din("sinT", [HD, s])                   # sign-folded sin[0].T
    wqT = din("wqT", [h, DQ], BF16)               # Wq[shard].T
    wkT = din("wkT", [h, HD], BF16)
    wvT = din("wvT", [h, HD], BF16)
    woT = din("woT", [DQ, h], BF16)               # Wo[:, shard].T
    ones_d = din("ones", [128, 128], F32R)
    onesb_d = din("ones_bf", [128, 1], BF16)
    ident_d = din("ident", [128, 128])
    out_d = nc.dram_tensor("out", [ql, h], BF16, kind="ExternalOutput").ap()

    with tile.TileContext(nc, trace_sim=trace_sim) as tc, ExitStack() as ctx:
        persist = ctx.enter_context(tc.tile_pool(name="persist", bufs=1))

        qr_sb = persist.tile([128, HPC, ql], BF16, tag="qr")    # [d, h, q]
        kr_sb = persist.tile([128, s], BF16, tag="kr")          # [d, k]
        v_sb = persist.tile([128, kt, 128], BF16, tag="v")      # [k%128, ktile, d]
        cos_sb = persist.tile([128, s], F32, tag="cos")
        sin_sb = persist.tile([128, s], F32, tag="sin")
        ones_sb = persist.tile([128, 128], F32R, tag="ones")
        onesb_sb = persist.tile([128, 1], BF16, tag="onesb")
        ident_sb = persist.tile([128, 128], F32, tag="ident")
        nc.sync.dma_start(cos_sb[:], cosT[:])
        nc.sync.dma_start(sin_sb[:], sinT[:])
        nc.sync.dma_start(ones_sb[:], ones_d[:])
        nc.sync.dma_start(onesb_sb[:], onesb_d[:])
        nc.sync.dma_start(ident_sb[:], ident_d[:])

        # ---------------- Phase A: projections + RoPE + V transpose ---------
        for _rep in range(body_reps):
          with (
              tc.tile_pool(name="wpool", bufs=1) as wpool,
              tc.tile_pool(name="apool", bufs=1) as apool,
              tc.tile_pool(name="psa", bufs=1, space=bass.MemorySpace.PSUM) as ps,
          ):
              wq_sb = wpool.tile([128, et, DQ], BF16, tag="wq")   # [e%128, etile, d]
              wk_sb = wpool.tile([128, et, HD], BF16, tag="wk")
              wv_sb = wpool.tile([128, et, HD], BF16, tag="wv")
              nc.sync.dma_start(
                  wq_sb[:], wqT.rearrange("(e p) d -> p e d", p=128)
              )
              nc.sync.dma_start(
                  wk_sb[:], wkT.rearrange("(e p) d -> p e d", p=128)
              )
              nc.sync.dma_start(
                  wv_sb[:], wvT.rearrange("(e p) d -> p e d", p=128)
              )

              def rope(ps_tile, q0c, q0s, dst):
                  # dst = ps*cos + rot_half(ps)*sin  (sin sign pre-folded)
                  # one fast ACT copy releases the PSUM bank; rotate-half via
                  # two partition-shifted ACT copies out of the SBUF staging
                  raw = apool.tile([128, QC], F32, tag="rraw", bufs=3)
                  nc.scalar.copy(raw[:], ps_tile[:])
                  swp = apool.tile([128, QC], F32, tag="rswp", bufs=3)
                  nc.scalar.copy(swp[0:64, :], raw[64:128, :])
                  nc.scalar.copy(swp[64:128, :], raw[0:64, :])
                  t1 = apool.tile([128, QC], F32, tag="rt1", bufs=2)
                  nc.vector.tensor_tensor(
                      t1[:], raw[:], cos_sb[:, q0c:q0c + QC], ALU.mult
                  )
                  t2 = apool.tile([128, QC], F32, tag="rt2", bufs=2)
                  nc.vector.tensor_tensor(
                      t2[:], swp[:], sin_sb[:, q0s:q0s + QC], ALU.mult
                  )
                  nc.vector.tensor_tensor(dst, t1[:], t2[:], ALU.add)

              for c in range(nch):
                  q0 = c * QC

                  psq = [
                      ps.tile([128, QC], F32, tag="ps", bufs=8, name=f"psq{i}")
                      for i in range(HPC)
                  ]
                  pskn = ps.tile([128, QC], F32, tag="ps", bufs=8)
                  pskc = ps.tile([128, QC], F32, tag="ps", bufs=8)
                  psvn = ps.tile([128, QC], F32, tag="ps", bufs=8)
                  psvc = ps.tile([128, QC], F32, tag="ps", bufs=8)

                  for e in range(et):
                      hs = apool.tile([128, QC], BF16, tag="hs", bufs=6)
                      nc.sync.dma_start(
                          hs[:], hiddenT[e * 128:e * 128 + 128, q0:q0 + QC]
                      )
                      ts_ = apool.tile([128, QC], BF16, tag="ts", bufs=6)
                      nc.sync.dma_start(
                          ts_[:], targetT[e * 128:e * 128 + 128, q0:q0 + QC]
                      )
                      st = dict(start=(e == 0), stop=(e == et - 1))
                      for hh in range(HPC):
                          nc.tensor.matmul(
                              psq[hh][:],
                              wq_sb[:, e, hh * 128:hh * 128 + 128],
                              hs[:],
                              **st,
                          )
                      nc.tensor.matmul(pskn[:], wk_sb[:, e, :], hs[:], **st)
                      nc.tensor.matmul(pskc[:], wk_sb[:, e, :], ts_[:], **st)
                      nc.tensor.matmul(psvn[:], wv_sb[:, e, :], hs[:], **st)
                      nc.tensor.matmul(psvc[:], wv_sb[:, e, :], ts_[:], **st)

                  # RoPE: Q and K_noise at positions ctx+q0.., K_ctx at q0..
                  for hh in range(HPC):
                      rope(psq[hh], ctx_len + q0, ctx_len + q0,
                           qr_sb[:, hh, q0:q0 + QC])
                  rope(pskn, ctx_len + q0, ctx_len + q0,
                       kr_sb[:, ctx_len + q0:ctx_len + q0 + QC])
                  rope(pskc, q0, q0, kr_sb[:, q0:q0 + QC])

                  # V: d-major [d, k] chunks -> PE transpose -> k-major bf16
                  for src, kbase in ((psvc, q0), (psvn, ctx_len + q0)):
                      vd = apool.tile([128, QC], F32, tag="vd", bufs=2)
                      nc.scalar.copy(vd[:], src[:])
                      for i in range(QC // 128):
                          pst = ps.tile([128, 128], F32, tag="ps", bufs=8)
                          nc.tensor.transpose(
                              pst[:], vd[:, i * 128:i * 128 + 128], ident_sb[:]
                          )
                          j = (kbase + i * 128) // 128
                          nc.vector.tensor_copy(v_sb[:, j, :], pst[:])

          # ---------------- Phase B/C: attention + output projection ----------
          with (
              tc.tile_pool(name="bpool", bufs=1) as bpool,
              tc.tile_pool(name="psb", bufs=1, space=bass.MemorySpace.PSUM) as ps,
          ):
              wo_sb = bpool.tile([128, HPC, h], BF16, tag="wo")
              nc.sync.dma_start(
                  wo_sb[:], woT.rearrange("(t p) o -> p t o", p=128)
              )

              at_sbs = {}
              stages = {}

              def emit_c_group(cqb, g):
                  # one Wo output group: 4 accumulating MMs + a DVE copy into
                  # the staging row; DMA the row out when it completes
                  qs, oc = divmod(g, hot)
                  if oc == 0:
                      stages[cqb, qs] = bpool.tile(
                          [128, hot, 512], BF16, tag="stage", bufs=2,
                          name=f"stage{cqb}_{qs}"
                      )
                  stage = stages[cqb, qs]
                  pso = ps.tile([128, 512], F32, tag="pso", bufs=1)
                  for t in range(HPC):
                      nc.tensor.matmul(
                          pso[:],
                          at_sbs[(cqb, t)][:, qs * 128:qs * 128 + 128],
                          wo_sb[:, t, oc * 512:oc * 512 + 512],
                          start=(t == 0),
                          stop=(t == HPC - 1),
                      )
                  nc.vector.tensor_copy(stage[:, oc, :], pso[:])
                  if oc == hot - 1:
                      r0 = cqb * QB + qs * 128
                      nc.sync.dma_start(out_d[r0:r0 + 128, :], stage[:])

              nc_qb = nqb if "B" in phases else 0
              do_c = "C" in phases
              for qb in range(nc_qb):
                  qs0 = qb * QB
                  # Wo-work of the previous block interleaves into this
                  # block's ACT-bound softmax loop (every other k-tile)
                  c_pend = list(range(4 * hot)) if (qb > 0 and do_c) else []
                  for pr in range(HPC // 2):
                      h0 = 2 * pr
                      expst = bpool.tile([128, kt, 2, QB], BF16,
                                         tag="expst", bufs=1)
                      psat = ps.tile([128, 2, QB], F32, tag="psat", bufs=1)
                      psrs = ps.tile([128, QB], F32, tag="psrs", bufs=1)
                      # software-pipelined: both heads' scores land in one
                      # 2-bank PSUM tile; a single wide exp covers the pair
                      LOOKAHEAD = 2

                      def emit_scores(j):
                          pss = ps.tile([128, 2, QB], F32, tag="pss", bufs=2,
                                        name=f"pss{j}")
                          for i in range(2):
                              nc.tensor.matmul(
                                  pss[:, i, :],
                                  kr_sb[:, j * 128:j * 128 + 128],
                                  qr_sb[:, h0 + i, qs0:qs0 + QB],
                                  start=True,
                                  stop=True,
                              )
                          nc.scalar.activation(
                              expst[:, j, :, :], pss[:, :, :], AF.Exp,
                              scale=scale
                          )

                      for j in range(min(LOOKAHEAD, kt)):
                          emit_scores(j)
                      for j in range(kt):
                          if j + LOOKAHEAD < kt:
                              emit_scores(j + LOOKAHEAD)
                          st = dict(start=(j == 0), stop=(j == kt - 1))
                          for i in range(2):
                              nc.tensor.matmul(
                                  psat[:, i, :],
                                  v_sb[:, j, :],
                                  expst[:, j, i, :],
                                  **st,
                              )
                              nc.tensor.matmul(
                                  psrs[32 * i:32 * i + 1, :],
                                  onesb_sb[:],
                                  expst[:, j, i, :],
                                  tile_position=(0, 32 * i),
                                  **st,
                              )
                          if j % 2 == 1 and c_pend:
                              emit_c_group(qb - 1, c_pend.pop(0))
                      # normalize: psb borrows a scores slot at pair end
                      psb = ps.tile([128, 2, QB], F32, tag="pss", bufs=2)
                      for i in range(2):
                          recip = bpool.tile([1, QB], F32R, tag="recip",
                                             bufs=4)
                          with nc.allow_low_precision(
                              reason="f32r reciprocal feeds the PE broadcast"
                          ):
                              nc.vector.reciprocal(
                                  recip[:], psrs[32 * i:32 * i + 1, :]
                              )
                          nc.tensor.matmul(
                              psb[:, i, :], ones_sb[0:1, :], recip[:],
                              start=True, stop=True,
                          )
                          at_raw = bpool.tile([128, QB], F32, tag="atraw",
                                              bufs=2)
                          nc.vector.tensor_copy(at_raw[:], psat[:, i, :])
                          at_sb = bpool.tile([128, QB], BF16, tag="attnT",
                                             bufs=8)
                          nc.vector.tensor_tensor(
                              at_sb[:], at_raw[:], psb[:, i, :], ALU.mult
                          )
                          at_sbs[(qb, h0 + i)] = at_sb
                  for g in c_pend:
                      emit_c_group(qb - 1, g)
              if nc_qb and do_c:
                  for g in range(4 * hot):
                      emit_c_group(nc_qb - 1, g)
    return _finish(nc)


def _finish(nc):
    nc.compile()
    return nc


def make_in_maps(hidden_states, target_hidden, cos, sin, Wq, Wk, Wv, Wo):
    hidden_states = np.asarray(hidden_states, dtype=np.float32)
    target_hidden = np.asarray(target_hidden, dtype=np.float32)
    cos = np.asarray(cos, dtype=np.float32)
    sin = np.asarray(sin, dtype=np.float32)
    Wq = np.asarray(Wq, dtype=np.float32)
    Wk = np.asarray(Wk, dtype=np.float32)
    Wv = np.asarray(Wv, dtype=np.float32)
    Wo = np.asarray(Wo, dtype=np.float32)

    bf16 = ml_dtypes.bfloat16
    hT = np.ascontiguousarray(hidden_states[0].T).astype(bf16)
    tT = np.ascontiguousarray(target_hidden[0].T).astype(bf16)
    cT = np.ascontiguousarray(cos[0].T)
    sT = np.ascontiguousarray(sin[0].T).copy()
    sT[:64, :] *= -1.0  # fold rotate_half sign: rot(x)*sin == swap(x)*sT
    ident = np.eye(128, dtype=np.float32)
    ones = np.ones((128, 128), dtype=np.float32)

    in_maps = []
    for c in range(NCORES):
        in_maps.append({
            "hiddenT": hT,
            "targetT": tT,
            "cosT": cT,
            "sinT": sT,
            "wqT": np.ascontiguousarray(Wq[512 * c:512 * c + 512, :].T).astype(bf16),
            "wkT": np.ascontiguousarray(Wk[128 * c:128 * c + 128, :].T).astype(bf16),
            "wvT": np.ascontiguousarray(Wv[128 * c:128 * c + 128, :].T).astype(bf16),
            "woT": np.ascontiguousarray(Wo[:, 512 * c:512 * c + 512].T).astype(bf16),
            "ones": ones,
            "ones_bf": np.ones((128, 1), dtype=bf16),
            "ident": ident,
        })
    return in_maps


_CACHE = {}
LAST_EXEC_NS = None
TRACE = False


def kernel(hidden_states, target_hidden, cos, sin, Wq, Wk, Wv, Wo):
    global LAST_EXEC_NS
    if "nc" not in _CACHE:
        _CACHE["nc"] = build_program()
    nc = _CACHE["nc"]
    in_maps = make_in_maps(
        hidden_states, target_hidden, cos, sin, Wq, Wk, Wv, Wo
    )
    res = run_bass_kernel_spmd(
        nc, in_maps, list(range(NCORES)), trace=TRACE
    )
    LAST_EXEC_NS = res.exec_time_ns
    out = np.zeros((QL, H), dtype=np.float32)
    for r in res.results:
        out += r["out"].astype(np.float32)
    return out.reshape(1, QL, H)


# revision 11
# speedup vs baseline: 1.0189x; 1.0189x over previous
"""DFlashAttention kernel for Trainium2, tensor-parallel across 8 NeuronCores.

Sharding: Megatron-style head parallelism. Core c owns KV head c and Q heads
4c..4c+3 (matches repeat_interleave grouping), i.e. Wq rows [512c, 512c+512),
Wk/Wv rows [128c, 128c+128), Wo columns [512c, 512c+512). Each core computes a
partial output [QL, H] in bf16; the host sums the 8 partials (row-parallel Wo).

v2 layout/schedule notes (all matmuls bf16 except the tiny f32r broadcast):
  - activations/weights fed feature-major (host pre-transposes)
  - Q/K kept d-major bf16 [HD, seq]; scores^T = K-tile.T @ Q per head
  - RoPE: rotate-half done with partition-shifted ACT copies straight out of
    PSUM (no SBUF<->SBUF DMA on the critical path); cos/sin resident in SBUF
  - softmax: per head-pair, both heads' score tiles land in one 2-bank PSUM
    tile and a single wide ACT exp (scale fused) emits bf16 probs for both;
    denominators accumulate on PE with a ones-column stationary at output
    partitions 0/32 (concurrent col-groups); normalization via reciprocal +
    PE broadcast and a DVE multiply
  - V kept k-major bf16 (PE-transposed after the d-major projection); PV
    accumulates attn^T = V^T @ P^T in PSUM over 32 k-tiles
  - Wo: attn^T tiles are the stationary operand; outputs assembled in a
    [128, 4096] bf16 staging row and written as one contiguous 1 MiB DMA on
    the ACT HWDGE ring (separate from the input ring)
"""

import math
from contextlib import ExitStack

import ml_dtypes
import numpy as np

import concourse.bass as bass
import concourse.bacc as bacc
import concourse.mybir as mybir
import concourse.tile as tile
from concourse.bass_utils import run_bass_kernel_spmd

F32 = mybir.dt.float32
F32R = mybir.dt.float32r
BF16 = mybir.dt.bfloat16
AF = mybir.ActivationFunctionType
ALU = mybir.AluOpType

# Full-problem dims (hardcoded per spec)
B, QL, CTX, H = 1, 2048, 2048, 4096
NH, NKV, HD = 32, 8, 128
NCORES = 8
HPC = NH // NKV  # 4 q-heads per core (one KV head per core)


def build_program(ql=QL, ctx_len=CTX, h=H, trace_sim=False, phases="ABC", body_reps=1):
    """Build the per-core Bass program (SPMD: same program, per-core shards)."""
    s = ql + ctx_len          # total kv length
    et = h // 128             # e-tiles (contraction tiles for projections)
    kt = s // 128             # k-tiles in attention
    QC = 512                  # phase A position-chunk
    nch = ql // QC            # chunks (ctx assumed == ql)
    assert ctx_len == ql, "phase A chunking assumes ctx_len == ql"
    QB = 512                  # phase B q-block
    nqb = ql // QB
    scale = 1.0 / math.sqrt(HD)
    DQ = HPC * HD             # 512: per-core q-head dim
    hot = h // 512            # output-column chunks in Wo stage

    nc = bacc.Bacc("TRN2", target_bir_lowering=False, debug=False)

    def din(name, shape, dt_=F32):
        return nc.dram_tensor(name, shape, dt_, kind="ExternalInput").ap()

    hiddenT = din("hiddenT", [h, ql], BF16)       # hidden_states[0].T
    targetT = din("targetT", [h, ctx_len], BF16)  # target_hidden[0].T
    cosT = din("cosT", [HD, s])                   # cos[0].T
    sinT = din("sinT", [HD, s])                   # sign-folded sin[0].T
    wqT = din("wqT", [h, DQ], BF16)               # Wq[shard].T
    wkT = din("wkT", [h, HD], BF16)
    wvT = din("wvT", [h, HD], BF16)
    woT = din("woT", [DQ, h], BF16)               # Wo[:, shard].T
    ones_d = din("ones", [128, 128], F32R)
    onesb_d = din("ones_bf", [128, 1], BF16)
    ident_d = din("ident", [128, 128])
    out_d = nc.dram_tensor("out", [ql, h], BF16, kind="ExternalOutput").ap()

    with tile.TileContext(nc, trace_sim=trace_sim) as tc, ExitStack() as ctx:
        persist = ctx.enter_context(tc.tile_pool(name="persist", bufs=1))

        qr_sb = persist.tile([128, HPC, ql], BF16, tag="qr")    # [d, h, q]
        kr_sb = persist.tile([128, s], BF16, tag="kr")          # [d, k]
        v_sb = persist.tile([128, kt, 128], BF16, tag="v")      # [k%128, ktile, d]
        cos_sb = persist.tile([128, s], F32, tag="cos")
        sin_sb = persist.tile([128, s], F32, tag="sin")
        ones_sb = persist.tile([128, 128], F32R, tag="ones")
        onesb_sb = persist.tile([128, 1], BF16, tag="onesb")
        ident_sb = persist.tile([128, 128], F32, tag="ident")
        nc.sync.dma_start(cos_sb[:], cosT[:])
        nc.sync.dma_start(sin_sb[:], sinT[:])
        nc.sync.dma_start(ones_sb[:], ones_d[:])
        nc.sync.dma_start(onesb_sb[:], onesb_d[:])
        nc.sync.dma_start(ident_sb[:], ident_d[:])

        # ---------------- Phase A: projections + RoPE + V transpose ---------
        for _rep in range(body_reps):
          with (
              tc.tile_pool(name="wpool", bufs=1) as wpool,
              tc.tile_pool(name="apool", bufs=1) as apool,
              tc.tile_pool(name="psa", bufs=1, space=bass.MemorySpace.PSUM) as ps,
          ):
              wq_sb = wpool.tile([128, et, DQ], BF16, tag="wq")   # [e%128, etile, d]
              wk_sb = wpool.tile([128, et, HD], BF16, tag="wk")
              wv_sb = wpool.tile([128, et, HD], BF16, tag="wv")
              nc.sync.dma_start(
                  wq_sb[:], wqT.rearrange("(e p) d -> p e d", p=128)
              )
              nc.sync.dma_start(
                  wk_sb[:], wkT.rearrange("(e p) d -> p e d", p=128)
              )
              nc.sync.dma_start(
                  wv_sb[:], wvT.rearrange("(e p) d -> p e d", p=128)
              )

              def rope(ps_tile, q0c, q0s, dst):
                  # dst = ps*cos + rot_half(ps)*sin  (sin sign pre-folded)
                  # one fast ACT copy releases the PSUM bank; rotate-half via
                  # two partition-shifted ACT copies out of the SBUF staging
                  raw = apool.tile([128, QC], F32, tag="rraw", bufs=3)
                  nc.scalar.copy(raw[:], ps_tile[:])
                  swp = apool.tile([128, QC], F32, tag="rswp", bufs=3)
                  nc.scalar.copy(swp[0:64, :], raw[64:128, :])
                  nc.scalar.copy(swp[64:128, :], raw[0:64, :])
                  t1 = apool.tile([128, QC], F32, tag="rt1", bufs=2)
                  nc.vector.tensor_tensor(
                      t1[:], raw[:], cos_sb[:, q0c:q0c + QC], ALU.mult
                  )
                  t2 = apool.tile([128, QC], F32, tag="rt2", bufs=2)
                  nc.vector.tensor_tensor(
                      t2[:], swp[:], sin_sb[:, q0s:q0s + QC], ALU.mult
                  )
                  nc.vector.tensor_tensor(dst, t1[:], t2[:], ALU.add)

              for c in range(nch):
                  q0 = c * QC

                  psq = [
                      ps.tile([128, QC], F32, tag="ps", bufs=8, name=f"psq{i}")
                      for i in range(HPC)
                  ]
                  pskn = ps.tile([128, QC], F32, tag="ps", bufs=8)
                  pskc = ps.tile([128, QC], F32, tag="ps", bufs=8)
                  psvn = ps.tile([128, QC], F32, tag="ps", bufs=8)
                  psvc = ps.tile([128, QC], F32, tag="ps", bufs=8)

                  for e in range(et):
                      hs = apool.tile([128, QC], BF16, tag="hs", bufs=6)
                      nc.sync.dma_start(
                          hs[:], hiddenT[e * 128:e * 128 + 128, q0:q0 + QC]
                      )
                      ts_ = apool.tile([128, QC], BF16, tag="ts", bufs=6)
                      nc.sync.dma_start(
                          ts_[:], targetT[e * 128:e * 128 + 128, q0:q0 + QC]
                      )
                      st = dict(start=(e == 0), stop=(e == et - 1))
                      for hh in range(HPC):
                          nc.tensor.matmul(
                              psq[hh][:],
                              wq_sb[:, e, hh * 128:hh * 128 + 128],
                              hs[:],
                              **st,
                          )
                      nc.tensor.matmul(pskn[:], wk_sb[:, e, :], hs[:], **st)
                      nc.tensor.matmul(pskc[:], wk_sb[:, e, :], ts_[:], **st)
                      nc.tensor.matmul(psvn[:], wv_sb[:, e, :], hs[:], **st)
                      nc.tensor.matmul(psvc[:], wv_sb[:, e, :], ts_[:], **st)

                  # RoPE: Q and K_noise at positions ctx+q0.., K_ctx at q0..
                  for hh in range(HPC):
                      rope(psq[hh], ctx_len + q0, ctx_len + q0,
                           qr_sb[:, hh, q0:q0 + QC])
                  rope(pskn, ctx_len + q0, ctx_len + q0,
                       kr_sb[:, ctx_len + q0:ctx_len + q0 + QC])
                  rope(pskc, q0, q0, kr_sb[:, q0:q0 + QC])

                  # V: d-major [d, k] chunks -> PE transpose -> k-major bf16
                  for src, kbase in ((psvc, q0), (psvn, ctx_len + q0)):
                      vd = apool.tile([128, QC], F32, tag="vd", bufs=2)
                      nc.scalar.copy(vd[:], src[:])
                      for i in range(QC // 128):
                          pst = ps.tile([128, 128], F32, tag="ps", bufs=8)
                          nc.tensor.transpose(
                              pst[:], vd[:, i * 128:i * 128 + 128], ident_sb[:]
                          )
                          j = (kbase + i * 128) // 128
                          nc.vector.tensor_copy(v_sb[:, j, :], pst[:])

          # ---------------- Phase B/C: attention + output projection ----------
          with (
              tc.tile_pool(name="bpool", bufs=1) as bpool,
              tc.tile_pool(name="psb", bufs=1, space=bass.MemorySpace.PSUM) as ps,
          ):
              wo_sb = bpool.tile([128, HPC, h], BF16, tag="wo")
              nc.sync.dma_start(
                  wo_sb[:], woT.rearrange("(t p) o -> p t o", p=128)
              )

              at_sbs = {}
              stages = {}

              def emit_c_group(cqb, g):
                  # one Wo output group: 4 accumulating MMs + a DVE copy into
                  # the staging row; DMA the row out when it completes
                  qs, oc = divmod(g, hot)
                  if oc == 0:
                      stages[cqb, qs] = bpool.tile(
                          [128, hot, 512], BF16, tag="stage", bufs=2,
                          name=f"stage{cqb}_{qs}"
                      )
                  stage = stages[cqb, qs]
                  pso = ps.tile([128, 512], F32, tag="pso", bufs=1)
                  for t in range(HPC):
                      nc.tensor.matmul(
                          pso[:],
                          at_sbs[(cqb, t)][:, qs * 128:qs * 128 + 128],
                          wo_sb[:, t, oc * 512:oc * 512 + 512],
                          start=(t == 0),
                          stop=(t == HPC - 1),
                      )
                  if "P" not in phases:
                      nc.vector.tensor_copy(stage[:, oc, :], pso[:])
                  if oc == hot - 1 and "P" not in phases and "D" not in phases:
                      r0 = cqb * QB + qs * 128
                      nc.sync.dma_start(out_d[r0:r0 + 128, :], stage[:])

              nc_qb = nqb if "B" in phases else 0
              do_c = "C" in phases
              for qb in range(nc_qb):
                  qs0 = qb * QB
                  # Wo-work of the previous block interleaves into this
                  # block's ACT-bound softmax loop (every other k-tile)
                  c_pend = []
                  if qb > 0 and do_c:
                      for g in range(4 * hot):
                          emit_c_group(qb - 1, g)
                  for pr in range(HPC // 2):
                      h0 = 2 * pr
                      expst = bpool.tile([128, kt, 2, QB], BF16,
                                         tag="expst", bufs=1)
                      psat = ps.tile([128, 2, QB], F32, tag="psat", bufs=1)
                      psrs = ps.tile([128, QB], F32, tag="psrs", bufs=1)
                      # software-pipelined: both heads' scores land in one
                      # 2-bank PSUM tile; a single wide exp covers the pair
                      LOOKAHEAD = 2

                      def emit_scores(j):
                          pss = ps.tile([128, 2, QB], F32, tag="pss", bufs=2,
                                        name=f"pss{j}")
                          for i in range(2):
                              nc.tensor.matmul(
                                  pss[:, i, :],
                                  kr_sb[:, j * 128:j * 128 + 128],
                                  qr_sb[:, h0 + i, qs0:qs0 + QB],
                                  start=True,
                                  stop=True,
                              )
                          nc.scalar.activation(
                              expst[:, j, :, :], pss[:, :, :], AF.Exp,
                              scale=scale
                          )

                      for j in range(min(LOOKAHEAD, kt)):
                          emit_scores(j)
                      for j in range(kt):
                          if j + LOOKAHEAD < kt:
                              emit_scores(j + LOOKAHEAD)
                          st = dict(start=(j == 0), stop=(j == kt - 1))
                          for i in range(2):
                              nc.tensor.matmul(
                                  psat[:, i, :],
                                  v_sb[:, j, :],
                                  expst[:, j, i, :],
                                  **st,
                              )
                              nc.tensor.matmul(
                                  psrs[32 * i:32 * i + 1, :],
                                  onesb_sb[:],
                                  expst[:, j, i, :],
                                  tile_position=(0, 32 * i),
                                  **st,
                              )
                          if j % 2 == 1 and c_pend:
                              emit_c_group(qb - 1, c_pend.pop(0))
                      # normalize: psb borrows a scores slot at pair end
                      psb = ps.tile([128, 2, QB], F32, tag="pss", bufs=2)
                      for i in range(2):
                          recip = bpool.tile([1, QB], F32R, tag="recip",
                                             bufs=4)
                          with nc.allow_low_precision(
                              reason="f32r reciprocal feeds the PE broadcast"
                          ):
                              nc.vector.reciprocal(
                                  recip[:], psrs[32 * i:32 * i + 1, :]
                              )
                          nc.tensor.matmul(
                              psb[:, i, :], ones_sb[0:1, :], recip[:],
                              start=True, stop=True,
                          )
                          at_raw = bpool.tile([128, QB], F32, tag="atraw",
                                              bufs=2)
                          nc.vector.tensor_copy(at_raw[:], psat[:, i, :])
                          at_sb = bpool.tile([128, QB], BF16, tag="attnT",
                                             bufs=8)
                          nc.vector.tensor_tensor(
                              at_sb[:], at_raw[:], psb[:, i, :], ALU.mult
                          )
                          at_sbs[(qb, h0 + i)] = at_sb
                  for g in c_pend:
                      emit_c_group(qb - 1, g)
              if nc_qb and do_c:
                  for g in range(4 * hot):
                      emit_c_group(nc_qb - 1, g)
    return _finish(nc)


def _finish(nc):
    nc.compile()
    return nc


def make_in_maps(hidden_states, target_hidden, cos, sin, Wq, Wk, Wv, Wo):
    hidden_states = np.asarray(hidden_states, dtype=np.float32)
    target_hidden = np.asarray(target_hidden, dtype=np.float32)
    cos = np.asarray(cos, dtype=np.float32)
    sin = np.asarray(sin, dtype=np.float32)
    Wq = np.asarray(Wq, dtype=np.float32)
    Wk = np.asarray(Wk, dtype=np.float32)
    Wv = np.asarray(Wv, dtype=np.float32)
    Wo = np.asarray(Wo, dtype=np.float32)

    bf16 = ml_dtypes.bfloat16
    hT = np.ascontiguousarray(hidden_states[0].T).astype(bf16)
    tT = np.ascontiguousarray(target_hidden[0].T).astype(bf16)
    cT = np.ascontiguousarray(cos[0].T)
    sT = np.ascontiguousarray(sin[0].T).copy()
    sT[:64, :] *= -1.0  # fold rotate_half sign: rot(x)*sin == swap(x)*sT
    ident = np.eye(128, dtype=np.float32)
    ones = np.ones((128, 128), dtype=np.float32)

    in_maps = []
    for c in range(NCORES):
        in_maps.append({
            "hiddenT": hT,
            "targetT": tT,
            "cosT": cT,
            "sinT": sT,
            "wqT": np.ascontiguousarray(Wq[512 * c:512 * c + 512, :].T).astype(bf16),
            "wkT": np.ascontiguousarray(Wk[128 * c:128 * c + 128, :].T).astype(bf16),
            "wvT": np.ascontiguousarray(Wv[128 * c:128 * c + 128, :].T).astype(bf16),
            "woT": np.ascontiguousarray(Wo[:, 512 * c:512 * c + 512].T).astype(bf16),
            "ones": ones,
            "ones_bf": np.ones((128, 1), dtype=bf16),
            "ident": ident,
        })
    return in_maps


_CACHE = {}
LAST_EXEC_NS = None
TRACE = False


def kernel(hidden_states, target_hidden, cos, sin, Wq, Wk, Wv, Wo):
    global LAST_EXEC_NS
    if "nc" not in _CACHE:
        _CACHE["nc"] = build_program()
    nc = _CACHE["nc"]
    in_maps = make_in_maps(
        hidden_states, target_hidden, cos, sin, Wq, Wk, Wv, Wo
    )
    res = run_bass_kernel_spmd(
        nc, in_maps, list(range(NCORES)), trace=TRACE
    )
    LAST_EXEC_NS = res.exec_time_ns
    out = np.zeros((QL, H), dtype=np.float32)
    for r in res.results:
        out += r["out"].astype(np.float32)
    return out.reshape(1, QL, H)


# revision 12
# speedup vs baseline: 1.0943x; 1.0741x over previous
"""DFlashAttention kernel for Trainium2, tensor-parallel across 8 NeuronCores.

Sharding: Megatron-style head parallelism. Core c owns KV head c and Q heads
4c..4c+3 (matches repeat_interleave grouping), i.e. Wq rows [512c, 512c+512),
Wk/Wv rows [128c, 128c+128), Wo columns [512c, 512c+512). Each core computes a
partial output [QL, H] in bf16; the host sums the 8 partials (row-parallel Wo).

v2 layout/schedule notes (all matmuls bf16 except the tiny f32r broadcast):
  - activations/weights fed feature-major (host pre-transposes)
  - Q/K kept d-major bf16 [HD, seq]; scores^T = K-tile.T @ Q per head
  - RoPE: rotate-half done with partition-shifted ACT copies straight out of
    PSUM (no SBUF<->SBUF DMA on the critical path); cos/sin resident in SBUF
  - softmax: per head-pair, both heads' score tiles land in one 2-bank PSUM
    tile and a single wide ACT exp (scale fused) emits bf16 probs for both;
    denominators accumulate on PE with a ones-column stationary at output
    partitions 0/32 (concurrent col-groups); normalization via reciprocal +
    PE broadcast and a DVE multiply
  - V kept k-major bf16 (PE-transposed after the d-major projection); PV
    accumulates attn^T = V^T @ P^T in PSUM over 32 k-tiles
  - Wo: attn^T tiles are the stationary operand; outputs assembled in a
    [128, 4096] bf16 staging row and written as one contiguous 1 MiB DMA on
    the ACT HWDGE ring (separate from the input ring)
"""

import math
from contextlib import ExitStack

import ml_dtypes
import numpy as np

import concourse.bass as bass
import concourse.bacc as bacc
import concourse.mybir as mybir
import concourse.tile as tile
from concourse.bass_utils import run_bass_kernel_spmd

F32 = mybir.dt.float32
F32R = mybir.dt.float32r
BF16 = mybir.dt.bfloat16
AF = mybir.ActivationFunctionType
ALU = mybir.AluOpType

# Full-problem dims (hardcoded per spec)
B, QL, CTX, H = 1, 2048, 2048, 4096
NH, NKV, HD = 32, 8, 128
NCORES = 8
HPC = NH // NKV  # 4 q-heads per core (one KV head per core)


def build_program(ql=QL, ctx_len=CTX, h=H, trace_sim=False, phases="ABC", body_reps=1):
    """Build the per-core Bass program (SPMD: same program, per-core shards)."""
    s = ql + ctx_len          # total kv length
    et = h // 128             # e-tiles (contraction tiles for projections)
    kt = s // 128             # k-tiles in attention
    QC = 512                  # phase A position-chunk
    nch = ql // QC            # chunks (ctx assumed == ql)
    assert ctx_len == ql, "phase A chunking assumes ctx_len == ql"
    QB = 512                  # phase B q-block
    nqb = ql // QB
    scale = 1.0 / math.sqrt(HD)
    DQ = HPC * HD             # 512: per-core q-head dim
    hot = h // 512            # output-column chunks in Wo stage

    nc = bacc.Bacc("TRN2", target_bir_lowering=False, debug=False)

    def din(name, shape, dt_=F32):
        return nc.dram_tensor(name, shape, dt_, kind="ExternalInput").ap()

    hiddenT = din("hiddenT", [h, ql], BF16)       # hidden_states[0].T
    targetT = din("targetT", [h, ctx_len], BF16)  # target_hidden[0].T
    cosT = din("cosT", [HD, s])                   # cos[0].T
    sinT = din("sinT", [HD, s])                   # sign-folded sin[0].T
    wqT = din("wqT", [h, DQ], BF16)               # Wq[shard].T
    wkT = din("wkT", [h, HD], BF16)
    wvT = din("wvT", [h, HD], BF16)
    woT = din("woT", [DQ, h], BF16)               # Wo[:, shard].T
    ones_d = din("ones", [128, 128], F32R)
    onesb_d = din("ones_bf", [128, 1], BF16)
    ident_d = din("ident", [128, 128])
    out_d = nc.dram_tensor("out", [ql, h], BF16, kind="ExternalOutput").ap()

    with tile.TileContext(nc, trace_sim=trace_sim) as tc, ExitStack() as ctx:
        persist = ctx.enter_context(tc.tile_pool(name="persist", bufs=1))

        qr_sb = persist.tile([128, HPC, ql], BF16, tag="qr")    # [d, h, q]
        kr_sb = persist.tile([128, s], BF16, tag="kr")          # [d, k]
        v_sb = persist.tile([128, kt, 128], BF16, tag="v")      # [k%128, ktile, d]
        cos_sb = persist.tile([128, s], F32, tag="cos")
        sin_sb = persist.tile([128, s], F32, tag="sin")
        ones_sb = persist.tile([128, 128], F32R, tag="ones")
        onesb_sb = persist.tile([128, 1], BF16, tag="onesb")
        ident_sb = persist.tile([128, 128], F32, tag="ident")
        nc.sync.dma_start(cos_sb[:], cosT[:])
        nc.sync.dma_start(sin_sb[:], sinT[:])
        nc.sync.dma_start(ones_sb[:], ones_d[:])
        nc.sync.dma_start(onesb_sb[:], onesb_d[:])
        nc.sync.dma_start(ident_sb[:], ident_d[:])

        # ---------------- Phase A: projections + RoPE + V transpose ---------
        for _rep in range(body_reps):
          with (
              tc.tile_pool(name="wpool", bufs=1) as wpool,
              tc.tile_pool(name="apool", bufs=1) as apool,
              tc.tile_pool(name="psa", bufs=1, space=bass.MemorySpace.PSUM) as ps,
          ):
              wq_sb = wpool.tile([128, et, DQ], BF16, tag="wq")   # [e%128, etile, d]
              wk_sb = wpool.tile([128, et, HD], BF16, tag="wk")
              wv_sb = wpool.tile([128, et, HD], BF16, tag="wv")
              nc.sync.dma_start(
                  wq_sb[:], wqT.rearrange("(e p) d -> p e d", p=128)
              )
              nc.sync.dma_start(
                  wk_sb[:], wkT.rearrange("(e p) d -> p e d", p=128)
              )
              nc.sync.dma_start(
                  wv_sb[:], wvT.rearrange("(e p) d -> p e d", p=128)
              )

              def rope(ps_tile, q0c, q0s, dst):
                  # dst = ps*cos + rot_half(ps)*sin  (sin sign pre-folded)
                  # one fast ACT copy releases the PSUM bank; rotate-half via
                  # two partition-shifted ACT copies out of the SBUF staging
                  raw = apool.tile([128, QC], F32, tag="rraw", bufs=3)
                  nc.scalar.copy(raw[:], ps_tile[:])
                  swp = apool.tile([128, QC], F32, tag="rswp", bufs=3)
                  nc.scalar.copy(swp[0:64, :], raw[64:128, :])
                  nc.scalar.copy(swp[64:128, :], raw[0:64, :])
                  t1 = apool.tile([128, QC], F32, tag="rt1", bufs=2)
                  nc.vector.tensor_tensor(
                      t1[:], raw[:], cos_sb[:, q0c:q0c + QC], ALU.mult
                  )
                  t2 = apool.tile([128, QC], F32, tag="rt2", bufs=2)
                  nc.vector.tensor_tensor(
                      t2[:], swp[:], sin_sb[:, q0s:q0s + QC], ALU.mult
                  )
                  nc.vector.tensor_tensor(dst, t1[:], t2[:], ALU.add)

              for c in range(nch):
                  q0 = c * QC

                  psq = [
                      ps.tile([128, QC], F32, tag="ps", bufs=8, name=f"psq{i}")
                      for i in range(HPC)
                  ]
                  pskn = ps.tile([128, QC], F32, tag="ps", bufs=8)
                  pskc = ps.tile([128, QC], F32, tag="ps", bufs=8)
                  psvn = ps.tile([128, QC], F32, tag="ps", bufs=8)
                  psvc = ps.tile([128, QC], F32, tag="ps", bufs=8)

                  for e in range(et):
                      hs = apool.tile([128, QC], BF16, tag="hs", bufs=6)
                      nc.sync.dma_start(
                          hs[:], hiddenT[e * 128:e * 128 + 128, q0:q0 + QC]
                      )
                      ts_ = apool.tile([128, QC], BF16, tag="ts", bufs=6)
                      nc.sync.dma_start(
                          ts_[:], targetT[e * 128:e * 128 + 128, q0:q0 + QC]
                      )
                      st = dict(start=(e == 0), stop=(e == et - 1))
                      for hh in range(HPC):
                          nc.tensor.matmul(
                              psq[hh][:],
                              wq_sb[:, e, hh * 128:hh * 128 + 128],
                              hs[:],
                              **st,
                          )
                      nc.tensor.matmul(pskn[:], wk_sb[:, e, :], hs[:], **st)
                      nc.tensor.matmul(pskc[:], wk_sb[:, e, :], ts_[:], **st)
                      nc.tensor.matmul(psvn[:], wv_sb[:, e, :], hs[:], **st)
                      nc.tensor.matmul(psvc[:], wv_sb[:, e, :], ts_[:], **st)

                  # RoPE: Q and K_noise at positions ctx+q0.., K_ctx at q0..
                  for hh in range(HPC):
                      rope(psq[hh], ctx_len + q0, ctx_len + q0,
                           qr_sb[:, hh, q0:q0 + QC])
                  rope(pskn, ctx_len + q0, ctx_len + q0,
                       kr_sb[:, ctx_len + q0:ctx_len + q0 + QC])
                  rope(pskc, q0, q0, kr_sb[:, q0:q0 + QC])

                  # V: d-major [d, k] chunks -> PE transpose -> k-major bf16
                  for src, kbase in ((psvc, q0), (psvn, ctx_len + q0)):
                      vd = apool.tile([128, QC], F32, tag="vd", bufs=2)
                      nc.scalar.copy(vd[:], src[:])
                      for i in range(QC // 128):
                          pst = ps.tile([128, 128], F32, tag="ps", bufs=8)
                          nc.tensor.transpose(
                              pst[:], vd[:, i * 128:i * 128 + 128], ident_sb[:]
                          )
                          j = (kbase + i * 128) // 128
                          nc.vector.tensor_copy(v_sb[:, j, :], pst[:])

          # ---------------- Phase B/C: attention + output projection ----------
          with (
              tc.tile_pool(name="bpool", bufs=1) as bpool,
              tc.tile_pool(name="psb", bufs=1, space=bass.MemorySpace.PSUM) as ps,
          ):
              wo_sb = bpool.tile([128, HPC, h], BF16, tag="wo")
              nc.sync.dma_start(
                  wo_sb[:], woT.rearrange("(t p) o -> p t o", p=128)
              )

              at_sbs = {}
              stages = {}

              c_state = {}

              def emit_c_group(cqb, g):
                  # one Wo output group: 4 accumulating MMs into a half-bank
                  # slot (two slots per PSUM bank -> groups pipeline 2-deep)
                  # + a DVE copy into the staging row; DMA when row completes
                  noc = 2 * hot           # 256-wide output chunks
                  qs, oc = divmod(g, noc)
                  if oc == 0:
                      stages[cqb, qs] = bpool.tile(
                          [128, noc, 256], BF16, tag="stage", bufs=2,
                          name=f"stage{cqb}_{qs}"
                      )
                  stage = stages[cqb, qs]
                  if g % 2 == 0:
                      c_state["pso"] = ps.tile(
                          [128, 2, 256], F32, tag="pso", bufs=1,
                          name=f"pso{cqb}_{g}"
                      )
                  pso = c_state["pso"]
                  sl = g % 2
                  for t in range(HPC):
                      nc.tensor.matmul(
                          pso[:, sl, :],
                          at_sbs[(cqb, t)][:, qs * 128:qs * 128 + 128],
                          wo_sb[:, t, oc * 256:oc * 256 + 256],
                          start=(t == 0),
                          stop=(t == HPC - 1),
                      )
                  if "P" not in phases:
                      nc.vector.tensor_copy(stage[:, oc, :], pso[:, sl, :])
                  if oc == noc - 1 and "P" not in phases and "D" not in phases:
                      r0 = cqb * QB + qs * 128
                      nc.sync.dma_start(out_d[r0:r0 + 128, :], stage[:])

              nc_qb = nqb if "B" in phases else 0
              do_c = "C" in phases
              for qb in range(nc_qb):
                  qs0 = qb * QB
                  # Wo-work of the previous block interleaves into this
                  # block's ACT-bound softmax loop (every other k-tile)
                  c_pend = list(range(8 * hot)) if (qb > 0 and do_c) else []
                  for pr in range(HPC // 2):
                      h0 = 2 * pr
                      expst = bpool.tile([128, kt, 2, QB], BF16,
                                         tag="expst", bufs=1)
                      psat = ps.tile([128, 2, QB], F32, tag="psat", bufs=1)
                      psrs = ps.tile([128, QB], F32, tag="psrs", bufs=1)
                      # software-pipelined: both heads' scores land in one
                      # 2-bank PSUM tile; a single wide exp covers the pair
                      LOOKAHEAD = 2

                      def emit_scores(j):
                          pss = ps.tile([128, 2, QB], F32, tag="pss", bufs=2,
                                        name=f"pss{j}")
                          for i in range(2):
                              nc.tensor.matmul(
                                  pss[:, i, :],
                                  kr_sb[:, j * 128:j * 128 + 128],
                                  qr_sb[:, h0 + i, qs0:qs0 + QB],
                                  start=True,
                                  stop=True,
                              )
                          nc.scalar.activation(
                              expst[:, j, :, :], pss[:, :, :], AF.Exp,
                              scale=scale
                          )

                      for j in range(min(LOOKAHEAD, kt)):
                          emit_scores(j)
                      for j in range(kt):
                          if j + LOOKAHEAD < kt:
                              emit_scores(j + LOOKAHEAD)
                          st = dict(start=(j == 0), stop=(j == kt - 1))
                          for i in range(2):
                              nc.tensor.matmul(
                                  psat[:, i, :],
                                  v_sb[:, j, :],
                                  expst[:, j, i, :],
                                  **st,
                              )
                              nc.tensor.matmul(
                                  psrs[32 * i:32 * i + 1, :],
                                  onesb_sb[:],
                                  expst[:, j, i, :],
                                  tile_position=(0, 32 * i),
                                  **st,
                              )
                          if c_pend:
                              emit_c_group(qb - 1, c_pend.pop(0))
                      # normalize: psb borrows a scores slot at pair end
                      psb = ps.tile([128, 2, QB], F32, tag="pss", bufs=2)
                      for i in range(2):
                          recip = bpool.tile([1, QB], F32R, tag="recip",
                                             bufs=4)
                          with nc.allow_low_precision(
                              reason="f32r reciprocal feeds the PE broadcast"
                          ):
                              nc.vector.reciprocal(
                                  recip[:], psrs[32 * i:32 * i + 1, :]
                              )
                          nc.tensor.matmul(
                              psb[:, i, :], ones_sb[0:1, :], recip[:],
                              start=True, stop=True,
                          )
                          at_raw = bpool.tile([128, QB], F32, tag="atraw",
                                              bufs=2)
                          nc.vector.tensor_copy(at_raw[:], psat[:, i, :])
                          at_sb = bpool.tile([128, QB], BF16, tag="attnT",
                                             bufs=8)
                          nc.vector.tensor_tensor(
                              at_sb[:], at_raw[:], psb[:, i, :], ALU.mult
                          )
                          at_sbs[(qb, h0 + i)] = at_sb
                  for g in c_pend:
                      emit_c_group(qb - 1, g)
              if nc_qb and do_c:
                  for g in range(8 * hot):
                      emit_c_group(nc_qb - 1, g)
    return _finish(nc)


def _finish(nc):
    nc.compile()
    return nc


def make_in_maps(hidden_states, target_hidden, cos, sin, Wq, Wk, Wv, Wo):
    hidden_states = np.asarray(hidden_states, dtype=np.float32)
    target_hidden = np.asarray(target_hidden, dtype=np.float32)
    cos = np.asarray(cos, dtype=np.float32)
    sin = np.asarray(sin, dtype=np.float32)
    Wq = np.asarray(Wq, dtype=np.float32)
    Wk = np.asarray(Wk, dtype=np.float32)
    Wv = np.asarray(Wv, dtype=np.float32)
    Wo = np.asarray(Wo, dtype=np.float32)

    bf16 = ml_dtypes.bfloat16
    hT = np.ascontiguousarray(hidden_states[0].T).astype(bf16)
    tT = np.ascontiguousarray(target_hidden[0].T).astype(bf16)
    cT = np.ascontiguousarray(cos[0].T)
    sT = np.ascontiguousarray(sin[0].T).copy()
    sT[:64, :] *= -1.0  # fold rotate_half sign: rot(x)*sin == swap(x)*sT
    ident = np.eye(128, dtype=np.float32)
    ones = np.ones((128, 128), dtype=np.float32)

    in_maps = []
    for c in range(NCORES):
        in_maps.append({
            "hiddenT": hT,
            "targetT": tT,
            "cosT": cT,
            "sinT": sT,
            "wqT": np.ascontiguousarray(Wq[512 * c:512 * c + 512, :].T).astype(bf16),
            "wkT": np.ascontiguousarray(Wk[128 * c:128 * c + 128, :].T).astype(bf16),
            "wvT": np.ascontiguousarray(Wv[128 * c:128 * c + 128, :].T).astype(bf16),
            "woT": np.ascontiguousarray(Wo[:, 512 * c:512 * c + 512].T).astype(bf16),
            "ones": ones,
            "ones_bf": np.ones((128, 1), dtype=bf16),
            "ident": ident,
        })
    return in_maps


_CACHE = {}
LAST_EXEC_NS = None
TRACE = False


def kernel(hidden_states, target_hidden, cos, sin, Wq, Wk, Wv, Wo):
    global LAST_EXEC_NS
    if "nc" not in _CACHE:
        _CACHE["nc"] = build_program()
    nc = _CACHE["nc"]
    in_maps = make_in_maps(
        hidden_states, target_hidden, cos, sin, Wq, Wk, Wv, Wo
    )
    res = run_bass_kernel_spmd(
        nc, in_maps, list(range(NCORES)), trace=TRACE
    )
    LAST_EXEC_NS = res.exec_time_ns
    out = np.zeros((QL, H), dtype=np.float32)
    for r in res.results:
        out += r["out"].astype(np.float32)
    return out.reshape(1, QL, H)


# revision 15
# speedup vs baseline: 1.1686x; 1.0678x over previous
"""DFlashAttention kernel for Trainium2, tensor-parallel across 8 NeuronCores.

Sharding: Megatron-style head parallelism. Core c owns KV head c and Q heads
4c..4c+3 (matches repeat_interleave grouping), i.e. Wq rows [512c, 512c+512),
Wk/Wv rows [128c, 128c+128), Wo columns [512c, 512c+512). Each core computes a
partial output [QL, H] in bf16; the host sums the 8 partials (row-parallel Wo).

Layout/schedule notes (all matmuls bf16 except the tiny f32r broadcast):
  - activations/weights fed feature-major (host pre-transposes)
  - Q/K kept d-major bf16 [HD, seq]; scores^T = K-tile.T @ Q per head
  - RoPE: rotate-half done with partition-shifted ACT copies straight out of
    PSUM (no SBUF<->SBUF DMA on the critical path); cos/sin resident in SBUF
  - softmax: per head-pair, both heads' score tiles land in one 2-bank PSUM
    tile and a single wide ACT exp (scale fused) emits bf16 probs for both;
    denominators accumulate on PE with a ones-column stationary at output
    partitions 0/32 (concurrent col-groups); normalization via reciprocal +
    PE broadcast and a DVE multiply
  - V kept k-major bf16 (PE-transposed after the d-major projection); PV
    accumulates attn^T = V^T @ P^T in PSUM over 32 k-tiles
  - Wo: attn^T tiles are the stationary operand; groups accumulate in
    half-bank PSUM slots (2-deep pipelined within one bank) and are
    interleaved one group per k-tile into the next q-block's ACT-bound
    softmax loop; outputs assembled in a [128, 4096] bf16 staging row and
    written as one contiguous 1 MiB DMA per 128-row block
"""

import math
from contextlib import ExitStack

import ml_dtypes
import numpy as np

import concourse.bass as bass
import concourse.bacc as bacc
import concourse.mybir as mybir
import concourse.tile as tile
from concourse.bass_utils import run_bass_kernel_spmd

F32 = mybir.dt.float32
F32R = mybir.dt.float32r
BF16 = mybir.dt.bfloat16
AF = mybir.ActivationFunctionType
ALU = mybir.AluOpType

# Full-problem dims (hardcoded per spec)
B, QL, CTX, H = 1, 2048, 2048, 4096
NH, NKV, HD = 32, 8, 128
NCORES = 8
HPC = NH // NKV  # 4 q-heads per core (one KV head per core)


def build_program(ql=QL, ctx_len=CTX, h=H, trace_sim=False, phases="ABC", body_reps=1):
    """Build the per-core Bass program (SPMD: same program, per-core shards)."""
    s = ql + ctx_len          # total kv length
    et = h // 128             # e-tiles (contraction tiles for projections)
    kt = s // 128             # k-tiles in attention
    QC = 512                  # phase A position-chunk
    nch = ql // QC            # chunks (ctx assumed == ql)
    assert ctx_len == ql, "phase A chunking assumes ctx_len == ql"
    QB = 512                  # phase B q-block
    nqb = ql // QB
    scale = 1.0 / math.sqrt(HD)
    DQ = HPC * HD             # 512: per-core q-head dim
    hot = h // 512            # output-column chunks in Wo stage

    nc = bacc.Bacc("TRN2", target_bir_lowering=False, debug=False)

    def din(name, shape, dt_=F32):
        return nc.dram_tensor(name, shape, dt_, kind="ExternalInput").ap()

    hiddenT = din("hiddenT", [h, ql], BF16)       # hidden_states[0].T
    targetT = din("targetT", [h, ctx_len], BF16)  # target_hidden[0].T
    cosT = din("cosT", [HD, s])                   # cos[0].T
    sinT = din("sinT", [HD, s])                   # sign-folded sin[0].T
    wqT = din("wqT", [h, DQ], BF16)               # Wq[shard].T
    wkT = din("wkT", [h, HD], BF16)
    wvT = din("wvT", [h, HD], BF16)
    woT = din("woT", [DQ, h], BF16)               # Wo[:, shard].T
    ones_d = din("ones", [128, 128], F32R)
    onesb_d = din("ones_bf", [128, 1], BF16)
    ident_d = din("ident", [128, 128])
    out_d = nc.dram_tensor("out", [ql, h], BF16, kind="ExternalOutput").ap()

    with tile.TileContext(nc, trace_sim=trace_sim) as tc, ExitStack() as ctx:
        persist = ctx.enter_context(tc.tile_pool(name="persist", bufs=1))

        qr_sb = persist.tile([128, HPC, ql], BF16, tag="qr")    # [d, h, q]
        kr_sb = persist.tile([128, s], BF16, tag="kr")          # [d, k]
        v_sb = persist.tile([128, kt, 128], BF16, tag="v")      # [k%128, ktile, d]
        cos_sb = persist.tile([128, s], F32, tag="cos")
        sin_sb = persist.tile([128, s], F32, tag="sin")
        ones_sb = persist.tile([128, 128], F32R, tag="ones")
        onesb_sb = persist.tile([128, 1], BF16, tag="onesb")
        ident_sb = persist.tile([128, 128], F32, tag="ident")
        nc.sync.dma_start(cos_sb[:], cosT[:])
        nc.sync.dma_start(sin_sb[:], sinT[:])
        nc.sync.dma_start(ones_sb[:], ones_d[:])
        nc.sync.dma_start(onesb_sb[:], onesb_d[:])
        nc.sync.dma_start(ident_sb[:], ident_d[:])

        # ---------------- Phase A: projections + RoPE + V transpose ---------
        for _rep in range(body_reps):
          with (
              tc.tile_pool(name="wpool", bufs=1) as wpool,
              tc.tile_pool(name="apool", bufs=1) as apool,
              tc.tile_pool(name="psa", bufs=1, space=bass.MemorySpace.PSUM) as ps,
          ):
              wq_sb = wpool.tile([128, et, DQ], BF16, tag="wq")   # [e%128, etile, d]
              wk_sb = wpool.tile([128, et, HD], BF16, tag="wk")
              wv_sb = wpool.tile([128, et, HD], BF16, tag="wv")
              nc.sync.dma_start(
                  wq_sb[:], wqT.rearrange("(e p) d -> p e d", p=128)
              )
              nc.sync.dma_start(
                  wk_sb[:], wkT.rearrange("(e p) d -> p e d", p=128)
              )
              nc.sync.dma_start(
                  wv_sb[:], wvT.rearrange("(e p) d -> p e d", p=128)
              )

              def rope(ps_tile, q0c, q0s, dst):
                  # dst = ps*cos + rot_half(ps)*sin  (sin sign pre-folded)
                  # one fast ACT copy releases the PSUM bank; rotate-half via
                  # two partition-shifted ACT copies out of the SBUF staging
                  raw = apool.tile([128, QC], F32, tag="rraw", bufs=3)
                  nc.scalar.copy(raw[:], ps_tile[:])
                  swp = apool.tile([128, QC], F32, tag="rswp", bufs=3)
                  nc.scalar.copy(swp[0:64, :], raw[64:128, :])
                  nc.scalar.copy(swp[64:128, :], raw[0:64, :])
                  t1 = apool.tile([128, QC], F32, tag="rt1", bufs=2)
                  nc.vector.tensor_tensor(
                      t1[:], raw[:], cos_sb[:, q0c:q0c + QC], ALU.mult
                  )
                  t2 = apool.tile([128, QC], F32, tag="rt2", bufs=2)
                  nc.vector.tensor_tensor(
                      t2[:], swp[:], sin_sb[:, q0s:q0s + QC], ALU.mult
                  )
                  nc.vector.tensor_tensor(dst, t1[:], t2[:], ALU.add)

              for c in range(nch):
                  q0 = c * QC

                  psq = [
                      ps.tile([128, QC], F32, tag="ps", bufs=8, name=f"psq{i}")
                      for i in range(HPC)
                  ]
                  pskn = ps.tile([128, QC], F32, tag="ps", bufs=8)
                  pskc = ps.tile([128, QC], F32, tag="ps", bufs=8)
                  psvn = ps.tile([128, QC], F32, tag="ps", bufs=8)
                  psvc = ps.tile([128, QC], F32, tag="ps", bufs=8)

                  for e in range(et):
                      hs = apool.tile([128, QC], BF16, tag="hs", bufs=6)
                      nc.sync.dma_start(
                          hs[:], hiddenT[e * 128:e * 128 + 128, q0:q0 + QC]
                      )
                      ts_ = apool.tile([128, QC], BF16, tag="ts", bufs=6)
                      nc.sync.dma_start(
                          ts_[:], targetT[e * 128:e * 128 + 128, q0:q0 + QC]
                      )
                      st = dict(start=(e == 0), stop=(e == et - 1))
                      for hh in range(HPC):
                          nc.tensor.matmul(
                              psq[hh][:],
                              wq_sb[:, e, hh * 128:hh * 128 + 128],
                              hs[:],
                              **st,
                          )
                      nc.tensor.matmul(pskn[:], wk_sb[:, e, :], hs[:], **st)
                      nc.tensor.matmul(pskc[:], wk_sb[:, e, :], ts_[:], **st)
                      nc.tensor.matmul(psvn[:], wv_sb[:, e, :], hs[:], **st)
                      nc.tensor.matmul(psvc[:], wv_sb[:, e, :], ts_[:], **st)

                  # RoPE: Q and K_noise at positions ctx+q0.., K_ctx at q0..
                  for hh in range(HPC):
                      rope(psq[hh], ctx_len + q0, ctx_len + q0,
                           qr_sb[:, hh, q0:q0 + QC])
                  rope(pskn, ctx_len + q0, ctx_len + q0,
                       kr_sb[:, ctx_len + q0:ctx_len + q0 + QC])
                  rope(pskc, q0, q0, kr_sb[:, q0:q0 + QC])

                  # V: d-major [d, k] chunks -> PE transpose -> k-major bf16
                  for src, kbase in ((psvc, q0), (psvn, ctx_len + q0)):
                      vd = apool.tile([128, QC], F32, tag="vd", bufs=2)
                      nc.scalar.copy(vd[:], src[:])
                      for i in range(QC // 128):
                          pst = ps.tile([128, 128], F32, tag="ps", bufs=8)
                          nc.tensor.transpose(
                              pst[:], vd[:, i * 128:i * 128 + 128], ident_sb[:]
                          )
                          j = (kbase + i * 128) // 128
                          nc.vector.tensor_copy(v_sb[:, j, :], pst[:])

          # ---------------- Phase B/C: attention + output projection ----------
          with (
              tc.tile_pool(name="bpool", bufs=1) as bpool,
              tc.tile_pool(name="psb", bufs=1, space=bass.MemorySpace.PSUM) as ps,
          ):
              wo_sb = bpool.tile([128, HPC, h], BF16, tag="wo")
              nc.sync.dma_start(
                  wo_sb[:], woT.rearrange("(t p) o -> p t o", p=128)
              )

              at_sbs = {}
              stages = {}

              c_state = {}

              def emit_c_group(cqb, g):
                  # one Wo output group: 4 accumulating MMs into a half-bank
                  # slot (two slots per PSUM bank -> groups pipeline 2-deep)
                  # + a DVE copy into the staging row; DMA when row completes
                  noc = 2 * hot           # 256-wide output chunks
                  qs, oc = divmod(g, noc)
                  if oc == 0:
                      stages[cqb, qs] = bpool.tile(
                          [128, noc, 256], BF16, tag="stage", bufs=2,
                          name=f"stage{cqb}_{qs}"
                      )
                  stage = stages[cqb, qs]
                  if g % 2 == 0:
                      c_state["pso"] = ps.tile(
                          [128, 2, 256], F32, tag="pso", bufs=1,
                          name=f"pso{cqb}_{g}"
                      )
                  pso = c_state["pso"]
                  sl = g % 2
                  for t in range(HPC):
                      nc.tensor.matmul(
                          pso[:, sl, :],
                          at_sbs[(cqb, t)][:, qs * 128:qs * 128 + 128],
                          wo_sb[:, t, oc * 256:oc * 256 + 256],
                          start=(t == 0),
                          stop=(t == HPC - 1),
                      )
                  if "P" not in phases:
                      nc.vector.tensor_copy(stage[:, oc, :], pso[:, sl, :])
                  if oc == noc - 1 and "P" not in phases and "D" not in phases:
                      r0 = cqb * QB + qs * 128
                      nc.sync.dma_start(out_d[r0:r0 + 128, :], stage[:])

              nc_qb = nqb if "B" in phases else 0
              do_c = "C" in phases
              for qb in range(nc_qb):
                  qs0 = qb * QB
                  # Wo-work of the previous block interleaves into this
                  # block's ACT-bound softmax loop (every other k-tile)
                  c_pend = list(range(8 * hot)) if (qb > 0 and do_c) else []
                  for pr in range(HPC // 2):
                      h0 = 2 * pr
                      expst = bpool.tile([128, kt, 2, QB], BF16,
                                         tag="expst", bufs=1)
                      psat = ps.tile([128, 2, QB], F32, tag="psat", bufs=1)
                      psrs = ps.tile([128, QB], F32, tag="psrs", bufs=1)
                      # software-pipelined: both heads' scores land in one
                      # 2-bank PSUM tile; a single wide exp covers the pair
                      LOOKAHEAD = 2

                      def emit_scores(j):
                          pss = ps.tile([128, 2, QB], F32, tag="pss", bufs=2,
                                        name=f"pss{j}")
                          for i in range(2):
                              nc.tensor.matmul(
                                  pss[:, i, :],
                                  kr_sb[:, j * 128:j * 128 + 128],
                                  qr_sb[:, h0 + i, qs0:qs0 + QB],
                                  start=True,
                                  stop=True,
                              )
                          nc.scalar.activation(
                              expst[:, j, :, :], pss[:, :, :], AF.Exp,
                              scale=scale
                          )

                      for j in range(min(LOOKAHEAD, kt)):
                          emit_scores(j)
                      for j in range(kt):
                          if j + LOOKAHEAD < kt:
                              emit_scores(j + LOOKAHEAD)
                          st = dict(start=(j == 0), stop=(j == kt - 1))
                          for i in range(2):
                              nc.tensor.matmul(
                                  psat[:, i, :],
                                  v_sb[:, j, :],
                                  expst[:, j, i, :],
                                  **st,
                              )
                              nc.tensor.matmul(
                                  psrs[32 * i:32 * i + 1, :],
                                  onesb_sb[:],
                                  expst[:, j, i, :],
                                  tile_position=(0, 32 * i),
                                  **st,
                              )
                          if c_pend:
                              emit_c_group(qb - 1, c_pend.pop(0))
                      # normalize: the reciprocal broadcast lands in the
                      # Wo-accumulator bank (two 256-wide halves) so the
                      # score slots free up for the next pair immediately
                      for i in range(2):
                          recip = bpool.tile([1, QB], F32R, tag="recip",
                                             bufs=4)
                          with nc.allow_low_precision(
                              reason="f32r reciprocal feeds the PE broadcast"
                          ):
                              nc.vector.reciprocal(
                                  recip[:], psrs[32 * i:32 * i + 1, :]
                              )
                          psb = ps.tile([128, 2, 256], F32, tag="pso",
                                        bufs=1, name=f"psb{qb}_{h0}_{i}")
                          for i2 in range(2):
                              nc.tensor.matmul(
                                  psb[:, i2, :], ones_sb[0:1, :],
                                  recip[:, i2 * 256:i2 * 256 + 256],
                                  start=True, stop=True,
                              )
                          at_raw = bpool.tile([128, QB], F32, tag="atraw",
                                              bufs=2)
                          nc.vector.tensor_copy(at_raw[:], psat[:, i, :])
                          at_sb = bpool.tile([128, QB], BF16, tag="attnT",
                                             bufs=8)
                          for i2 in range(2):
                              nc.vector.tensor_tensor(
                                  at_sb[:, i2 * 256:i2 * 256 + 256],
                                  at_raw[:, i2 * 256:i2 * 256 + 256],
                                  psb[:, i2, :], ALU.mult
                              )
                          at_sbs[(qb, h0 + i)] = at_sb
                  for g in c_pend:
                      emit_c_group(qb - 1, g)
              if nc_qb and do_c:
                  for g in range(8 * hot):
                      emit_c_group(nc_qb - 1, g)
    return _finish(nc)


def _finish(nc):
    nc.compile()
    return nc


def make_in_maps(hidden_states, target_hidden, cos, sin, Wq, Wk, Wv, Wo):
    hidden_states = np.asarray(hidden_states, dtype=np.float32)
    target_hidden = np.asarray(target_hidden, dtype=np.float32)
    cos = np.asarray(cos, dtype=np.float32)
    sin = np.asarray(sin, dtype=np.float32)
    Wq = np.asarray(Wq, dtype=np.float32)
    Wk = np.asarray(Wk, dtype=np.float32)
    Wv = np.asarray(Wv, dtype=np.float32)
    Wo = np.asarray(Wo, dtype=np.float32)

    bf16 = ml_dtypes.bfloat16
    hT = np.ascontiguousarray(hidden_states[0].T).astype(bf16)
    tT = np.ascontiguousarray(target_hidden[0].T).astype(bf16)
    cT = np.ascontiguousarray(cos[0].T)
    sT = np.ascontiguousarray(sin[0].T).copy()
    sT[:64, :] *= -1.0  # fold rotate_half sign: rot(x)*sin == swap(x)*sT
    ident = np.eye(128, dtype=np.float32)
    ones = np.ones((128, 128), dtype=np.float32)

    in_maps = []
    for c in range(NCORES):
        in_maps.append({
            "hiddenT": hT,
            "targetT": tT,
            "cosT": cT,
            "sinT": sT,
            "wqT": np.ascontiguousarray(Wq[512 * c:512 * c + 512, :].T).astype(bf16),
            "wkT": np.ascontiguousarray(Wk[128 * c:128 * c + 128, :].T).astype(bf16),
            "wvT": np.ascontiguousarray(Wv[128 * c:128 * c + 128, :].T).astype(bf16),
            "woT": np.ascontiguousarray(Wo[:, 512 * c:512 * c + 512].T).astype(bf16),
            "ones": ones,
            "ones_bf": np.ones((128, 1), dtype=bf16),
            "ident": ident,
        })
    return in_maps


_CACHE = {}
LAST_EXEC_NS = None
TRACE = False


def kernel(hidden_states, target_hidden, cos, sin, Wq, Wk, Wv, Wo):
    global LAST_EXEC_NS
    if "nc" not in _CACHE:
        _CACHE["nc"] = build_program()
    nc = _CACHE["nc"]
    in_maps = make_in_maps(
        hidden_states, target_hidden, cos, sin, Wq, Wk, Wv, Wo
    )
    res = run_bass_kernel_spmd(
        nc, in_maps, list(range(NCORES)), trace=TRACE
    )
    LAST_EXEC_NS = res.exec_time_ns
    out = np.zeros((QL, H), dtype=np.float32)
    for r in res.results:
        out += r["out"].astype(np.float32)
    return out.reshape(1, QL, H)
